# revision 1
# baseline (speedup 1.0000x reference)
"""CausalWanSelfAttention on 8 Trainium2 NeuronCores.

Sharding: 16 query chunks of 252 rows; core c owns chunks (c, 15-c) for
causal load balance.  Each core projects q/k/v for its own 504 rows,
ropes+norms them, AllGathers K (transposed) and V (rows) across the 8
cores, runs frame-causal flash attention for its queries over all keys,
and projects the output rows.  Host assembles the full [1, 4032, 1536]
output from per-core row slices.
"""
import sys, os, types, math

sys.path.insert(0, "/opt/trn_rl_repo")

import numpy as np

import concourse.bass as bass
import concourse.tile as tile
from concourse import mybir
from concourse.masks import make_identity

F32 = mybir.dt.float32
F32R = mybir.dt.float32r

# Static problem geometry
S, D, H, HD = 4032, 1536, 12, 128
F_FRAMES, H_GRID, W_GRID = 9, 16, 28
FRAME = H_GRID * W_GRID            # 448
ROT = HD // 2                      # 64
EPS = 1e-6
SCALE = 1.0 / math.sqrt(HD)

NC = 8                             # cores
NCHUNK = 16                        # query chunks
CH = S // NCHUNK                   # 252 rows per chunk
KT = 126                           # k-tile rows (2 per chunk)
NKT = S // KT                      # 32 k-tiles
ST = 126                           # q row-tile (4 per core)
NST = 4
ROWS = 2 * CH                      # 504 own rows per core
CB = 512                           # projection column block
NCB = D // CB                      # 3
NKC = D // 128                     # 12 contraction chunks

# chunk pair owned by core c: (c, 15-c)
def core_chunks(c):
    return (c, NCHUNK - 1 - c)

# global row range of chunk j
def chunk_rows(j):
    return CH * j, CH * (j + 1)

# k-tile kt (global row order) -> (source core, col offset within its 504)
def kt_src(kt):
    j = kt // 2
    src = j if j < NC else NCHUNK - 1 - j
    sub = 0 if j < NC else 1
    return src, sub * CH + (kt % 2) * KT

# static per-qb k-loop bounds (tiles of KT), max over cores
def _bounds():
    bnd = []
    for qb in range(2):
        mx = 0
        for c in range(NC):
            j = core_chunks(c)[qb]
            last_row = CH * j + CH - 1
            keys = (last_row // FRAME + 1) * FRAME
            mx = max(mx, -(-keys // KT))
        bnd.append(mx)
    return bnd

BND = _bounds()                    # [18, 32]

# which k-tiles are all-allowed for every core (no mask needed)
def _clean_tiles():
    clean = []
    for qb in range(2):
        mn = S + 1
        for c in range(NC):
            j = core_chunks(c)[qb]
            first_row = CH * j
            keys = (first_row // FRAME + 1) * FRAME
            mn = min(mn, keys)
        clean.append(mn // KT)     # tiles [0, clean) need no mask
    return clean

CLEAN = _clean_tiles()             # [3, 17]

# masked tile-PAIRS per qb: pair p covers k-tiles (2p, 2p+1)
def masked_pairs(qb):
    lo, hi = CLEAN[qb], BND[qb]
    pairs = sorted({kt // 2 for kt in range(lo, hi)})
    return pairs

MP0, MP1 = masked_pairs(0), masked_pairs(1)
N_MASKS = len(MP0) + len(MP1)

DMA_HOIST_SKIP = {"InstCall", "InstNoOp"}


def fix_waits(nc):
    """This container's walrus rejects any instruction carrying more
    than one sync wait.  Hoist waits onto preceding NoOps (one wait per
    NoOp) on the same engine; sequencer program order preserves the
    semantics."""
    nop_id = [0]
    for f in nc.m.functions:
        for blk in f.blocks:
            out, changed = [], False
            for inst in blk.instructions:
                tname = type(inst).__name__
                si = inst.sync_info
                if (tname not in DMA_HOIST_SKIP and si is not None
                        and len(si.on_wait) > 1):
                    for w in list(si.on_wait):
                        nop = mybir.InstNoOp(name=f"I-waitnop-{nop_id[0]}")
                        nop_id[0] += 1
                        nop.engine = inst.engine
                        nop.sync_info = mybir.SyncInfo(on_wait=[w], on_update=[])
                        out.append(nop)
                    si.on_wait = []
                    changed = True
                out.append(inst)
            if changed:
                blk.instructions = out


def build_program():
    nc = bass.Bass(num_devices=NC)

    x_own = nc.dram_tensor("x_own", [ROWS, D], F32, kind="ExternalInput")
    Wq = nc.dram_tensor("Wq", [D, D], F32, kind="ExternalInput")
    Wk = nc.dram_tensor("Wk", [D, D], F32, kind="ExternalInput")
    Wv = nc.dram_tensor("Wv", [D, D], F32, kind="ExternalInput")
    Wo = nc.dram_tensor("Wo", [D, D], F32, kind="ExternalInput")
    bq = nc.dram_tensor("bq", [D], F32, kind="ExternalInput")
    bk = nc.dram_tensor("bk", [D], F32, kind="ExternalInput")
    bv = nc.dram_tensor("bv", [D], F32, kind="ExternalInput")
    bo = nc.dram_tensor("bo", [D], F32, kind="ExternalInput")
    gq = nc.dram_tensor("gq", [D], F32, kind="ExternalInput")
    gk = nc.dram_tensor("gk", [D], F32, kind="ExternalInput")
    cosf = nc.dram_tensor("cosf", [ROWS, HD], F32, kind="ExternalInput")
    sinf = nc.dram_tensor("sinf", [ROWS, HD], F32, kind="ExternalInput")
    masks = nc.dram_tensor("masks", [max(N_MASKS, 1), KT, 2 * CH], F32,
                           kind="ExternalInput")
    out_own = nc.dram_tensor("out_own", [ROWS, D], F32, kind="ExternalOutput")

    kT_bounce = nc.dram_tensor("kT_bounce", [D, ROWS], F32)
    v_bounce = nc.dram_tensor("v_bounce", [ROWS, D], F32)
    kT_g = nc.dram_tensor("kT_g", [NC, D, ROWS], F32, addr_space="Shared")
    v_g = nc.dram_tensor("v_g", [NC, ROWS, D], F32, addr_space="Shared")
    recip_scratch = nc.dram_tensor("recip_scratch", [H, 2, CH], F32)

    with tile.TileContext(nc) as tc:
        with tc.tile_pool(name="persist", bufs=1) as persist:
            # ---- constants ----
            ident = persist.tile([128, 128], F32)
            make_identity(nc, ident[:])
            ones_f = persist.tile([128, 128], F32)
            nc.gpsimd.memset(ones_f[:], 1.0)
            ones_r = persist.tile([128, 128], F32)
            nc.vector.tensor_copy(ones_r[:].bitcast(F32R), ones_f[:])
            eps_c = persist.tile([128, 1], F32)
            nc.gpsimd.memset(eps_c[:], EPS)
            rcpD_c = persist.tile([128, 1], F32)
            nc.gpsimd.memset(rcpD_c[:], 1.0 / D)
            scl_c = persist.tile([128, 1], F32)
            nc.gpsimd.memset(scl_c[:], SCALE)



            qT = persist.tile([128, H, ROWS], F32)       # q transposed, f32r content

            # ---- phase 1: x rows -> xT ----
            with (
                tc.tile_pool(name="p1", bufs=2) as p1,
                tc.tile_pool(name="xTp", bufs=1) as xTp,
                tc.tile_pool(name="gpool", bufs=1) as gpool,
                tc.tile_pool(name="bpool", bufs=2) as bpool,
                tc.tile_pool(name="ps_tp", bufs=2, space="PSUM") as ps_tp,
                tc.tile_pool(name="ps_acc", bufs=1, space="PSUM") as ps_acc,
            ):
                cs_sb = gpool.tile([ST, 2, NST, HD], F32, tag="cs")
                nc.sync.dma_start(out=cs_sb[:, 0],
                                  in_=cosf.rearrange("(t p) f -> p t f", p=ST))
                nc.sync.dma_start(out=cs_sb[:, 1],
                                  in_=sinf.rearrange("(t p) f -> p t f", p=ST))
                gq_b = gpool.tile([128, D], F32, tag="gq")
                nc.sync.dma_start(out=gq_b[:], in_=gq[None, :].partition_broadcast(128))
                gk_b = gpool.tile([128, D], F32, tag="gk")
                nc.sync.dma_start(out=gk_b[:], in_=gk[None, :].partition_broadcast(128))

                def bias_slice(b, cb):
                    t = bpool.tile([1, CB], F32, tag="bias")
                    nc.sync.dma_start(
                        out=t[:].bitcast(F32R),
                        in_=b[None, cb * CB:(cb + 1) * CB].bitcast(F32R))
                    return t

                xT = xTp.tile([128, NKC, ROWS], F32)     # f32r content
                for st in range(NST):
                    xr = p1.tile([ST, D], F32, tag="xrow")
                    nc.sync.dma_start(out=xr[:], in_=x_own[st * ST:(st + 1) * ST, :])
                    for kc in range(NKC):
                        pt = ps_tp.tile([128, ST], F32, tag="tp")
                        nc.tensor.transpose(pt[:], xr[:, kc * 128:(kc + 1) * 128], ident[0:ST, 0:ST])
                        nc.vector.tensor_copy(
                            xT[:, kc, st * ST:(st + 1) * ST].bitcast(F32R), pt[:])

                # ---- phase 2: projections ----
                def project(W, bias_idx, out_cb):
                    """h = x @ W + b per s-tile/c-block; out_cb(st, cb, acc)
                    consumes the PSUM tile."""
                    with tc.tile_pool(name="wpool", bufs=2) as wp:
                        for cb in range(NCB):
                            wt = wp.tile([128, NKC, CB], F32, tag="w")
                            nc.sync.dma_start(
                                out=wt[:].bitcast(F32R),
                                in_=W[:, cb * CB:(cb + 1) * CB]
                                .rearrange("(k p) c -> p k c", p=128).bitcast(F32R))
                            for st in range(NST):
                                acc = ps_acc.tile([ST, CB], F32, tag=f"acc{st % 2}")
                                for kc in range(NKC):
                                    nc.tensor.matmul(
                                        acc[:],
                                        xT[:, kc, st * ST:(st + 1) * ST].bitcast(F32R),
                                        wt[:, kc, :].bitcast(F32R),
                                        start=(kc == 0), stop=False)
                                nc.tensor.matmul(
                                    acc[:], ones_r[0:1, 0:ST].bitcast(F32R),
                                    bias_slice(bias_idx, cb)[:].bitcast(F32R),
                                    start=False, stop=True)
                                out_cb(st, cb, acc)

                # --- v: no norm; straight to bounce ---
                def v_out(st, cb, acc):
                    vr = p1.tile([ST, CB], F32, tag="vrow")
                    nc.scalar.activation(vr[:], acc[:],
                                         mybir.ActivationFunctionType.Copy)
                    nc.sync.dma_start(
                        out=v_bounce[st * ST:(st + 1) * ST, cb * CB:(cb + 1) * CB],
                        in_=vr[:])

                project(Wv, bv, v_out)

                # --- q/k: rmsnorm + gain + rope, then transpose ---
                def norm_rope(W, bias, g_b, sink):
                    """sink(kc, st, tile128x126_f32r_psum) consumes transposed
                    normalized+roped tiles."""
                    with (
                        tc.tile_pool(name="nr", bufs=2) as nr,
                        tc.tile_pool(name="nr1", bufs=1) as nr1,
                    ):
                        rows_all = nr1.tile([ST, NST, D], F32)
                        var_p = nr1.tile([ST, NST, NCB], F32)
                        # pass A: h = x@W + b -> rows_all; sumsq -> var_p
                        for cb in range(NCB):
                            wt = nr.tile([128, NKC, CB], F32, tag="w2")
                            nc.sync.dma_start(
                                out=wt[:].bitcast(F32R),
                                in_=W[:, cb * CB:(cb + 1) * CB]
                                .rearrange("(k p) c -> p k c", p=128).bitcast(F32R))
                            for st in range(NST):
                                acc = ps_acc.tile([ST, CB], F32, tag=f"acc{st % 2}")
                                for kc in range(NKC):
                                    nc.tensor.matmul(
                                        acc[:],
                                        xT[:, kc, st * ST:(st + 1) * ST].bitcast(F32R),
                                        wt[:, kc, :].bitcast(F32R),
                                        start=(kc == 0), stop=False)
                                nc.tensor.matmul(
                                    acc[:], ones_r[0:1, 0:ST].bitcast(F32R),
                                    bias_slice(bias, cb)[:].bitcast(F32R),
                                    start=False, stop=True)
                                sq = nr.tile([ST, CB], F32, tag="sq")
                                nc.scalar.activation(
                                    sq[:], acc[:],
                                    mybir.ActivationFunctionType.Square,
                                    accum_out=var_p[:, st, cb:cb + 1])
                                nc.vector.tensor_copy(
                                    rows_all[:, st, cb * CB:(cb + 1) * CB], acc[:])
                        # pass B: rstd, normalize, gain, rope, transpose
                        for st in range(NST):
                            var = nr.tile([ST, 1], F32, tag="var")
                            nc.vector.reduce_sum(var[:], var_p[:, st, :],
                                                 axis=mybir.AxisListType.X)
                            sdt = nr.tile([ST, 1], F32, tag="sd")
                            nc.scalar.activation(
                                sdt[:], var[:], mybir.ActivationFunctionType.Sqrt,
                                bias=eps_c[0:ST, :], scale=rcpD_c[0:ST, :])
                            rstd = nr.tile([ST, 1], F32, tag="rstd")
                            nc.vector.reciprocal(rstd[:], sdt[:])
                            rows = nr.tile([ST, D], F32, tag="rows")
                            nc.scalar.activation(
                                rows[:], rows_all[:, st, :],
                                mybir.ActivationFunctionType.Copy,
                                scale=rstd[:])
                            nc.vector.tensor_mul(rows[:], rows[:], g_b[0:ST, :])
                            # rope: out = rows*cosF + swap(rows)*sinF
                            swap = nr.tile([ST, D], F32, tag="swap")
                            r3 = rows[:].rearrange("p (h r two) -> p h r two",
                                                   two=2, r=ROT)
                            s3 = swap[:].rearrange("p (h r two) -> p h r two",
                                                   two=2, r=ROT)
                            nc.vector.tensor_copy(s3[:, :, :, 0], r3[:, :, :, 1])
                            nc.vector.tensor_copy(s3[:, :, :, 1], r3[:, :, :, 0])
                            cosb = (cs_sb[:, 0, st, :].unsqueeze(1)
                                    .broadcast_to((ST, H, HD)))
                            sinb = (cs_sb[:, 1, st, :].unsqueeze(1)
                                    .broadcast_to((ST, H, HD)))
                            rr = rows[:].rearrange("p (h f) -> p h f", h=H)
                            sr = swap[:].rearrange("p (h f) -> p h f", h=H)
                            nc.vector.tensor_mul(rr[:], rr[:], cosb)
                            nc.vector.tensor_mul(sr[:], sr[:], sinb)
                            nc.vector.tensor_add(rows[:], rows[:], swap[:])
                            for kc in range(NKC):
                                pt = ps_tp.tile([128, ST], F32, tag="tp")
                                nc.tensor.transpose(
                                    pt[:], rows[:, kc * 128:(kc + 1) * 128],
                                    ident[0:ST, 0:ST])
                                sink(kc, st, pt)

                def q_sink(kc, st, pt):
                    nc.vector.tensor_copy(
                        qT[:, kc, st * ST:(st + 1) * ST].bitcast(F32R), pt[:])

                ktmp_pool = tc.tile_pool(name="ktmp", bufs=3)
                ktmp = ktmp_pool.__enter__()

                kstage = {}

                def k_sink(kc, st, pt):
                    if st not in kstage:
                        kstage[st] = ktmp.tile([128, NKC, ST], F32, tag="ktr", name=f"kstage{st}")
                    kt_sb = kstage[st]
                    nc.vector.tensor_copy(kt_sb[:, kc, :], pt[:])
                    if kc == NKC - 1:
                        nc.sync.dma_start(
                            out=kT_bounce[:, st * ST:(st + 1) * ST]
                            .rearrange("(k p) s -> p k s", p=128),
                            in_=kt_sb[:])
                        del kstage[st]

                norm_rope(Wk, bk, gk_b, k_sink)
                nc.gpsimd.collective_compute(
                    "AllGather", mybir.AluOpType.bypass,
                    replica_groups=[list(range(NC))],
                    ins=[kT_bounce[:].opt()], outs=[kT_g[:].opt()])
                norm_rope(Wq, bq, gq_b, q_sink)
                ktmp_pool.__exit__(None, None, None)

            # ---- phase 3: allgather V (K gathered above) ----
            nc.gpsimd.collective_compute(
                "AllGather", mybir.AluOpType.bypass,
                replica_groups=[list(range(NC))],
                ins=[v_bounce[:].opt()], outs=[v_g[:].opt()])

            # ---- phase 4: attention ----
            with tc.tile_pool(name="late", bufs=1) as late:
              attnT = late.tile([128, H, ROWS], F32)   # attention out^T, f32r
              with (
                tc.tile_pool(name="att", bufs=2) as att,
                tc.tile_pool(name="mk", bufs=1) as mk,
                tc.tile_pool(name="psO", bufs=2, space="PSUM") as psO,
                tc.tile_pool(name="ps_s", bufs=2, space="PSUM") as ps_s_pool,
            ):
                mask_sb = mk.tile([KT, max(N_MASKS, 1), 2 * CH], F32)
                nc.sync.dma_start(out=mask_sb[:],
                                  in_=masks.rearrange("n p q -> p n q"))
                # mask index lookup: (qb, pair) -> slot
                mask_slot = {}
                for p in MP0:
                    mask_slot[(0, p)] = len(mask_slot)
                for p in MP1:
                    mask_slot[(1, p)] = len(mask_slot)

                for h in range(int(os.environ.get('WAN_ATT_H', str(H)))):
                    kT_h = att.tile([128, NC, ROWS], F32, tag="kTh")
                    nc.sync.dma_start(
                        out=kT_h[:].bitcast(F32R),
                        in_=kT_g[:, h * HD:(h + 1) * HD, :]
                        .rearrange("s p c -> p s c").bitcast(F32R))
                    v_h = att.tile([KT, NC * 4, HD], F32, tag="vh")
                    for src in range(NC):
                        nc.sync.dma_start(
                            out=v_h[:, src * 4:(src + 1) * 4, :].bitcast(F32R),
                            in_=v_g[src, :, h * HD:(h + 1) * HD]
                            .rearrange("(r p) d -> p r d", p=KT).bitcast(F32R))
                    for qb in range(2):
                        ps_o = psO.tile([128, CH], F32, tag="o")
                        ps_den = (None if os.environ.get("WAN_NO_DEN")
                                  else psO.tile([1, CH], F32, tag="den"))
                        npair = BND[qb] // 2
                        for pr in range(npair):
                            ps_s = ps_s_pool.tile([KT, 2, CH], F32, tag="s")
                            for half in range(2):
                                kt = 2 * pr + half
                                src, coff = kt_src(kt)
                                nc.tensor.matmul(
                                    ps_s[:, half, :],
                                    kT_h[:, src, coff:coff + KT].bitcast(F32R),
                                    qT[:, h, qb * CH:(qb + 1) * CH].bitcast(F32R),
                                    start=True, stop=True)
                            expT = att.tile([KT, 2, CH], F32, tag="expT")
                            nc.scalar.activation(
                                expT[:].bitcast(F32R), ps_s[:],
                                mybir.ActivationFunctionType.Exp,
                                scale=scl_c[0:KT, :])
                            slot = mask_slot.get((qb, pr))
                            if slot is not None:
                                nc.vector.tensor_mul(
                                    expT[:].bitcast(F32R), expT[:].bitcast(F32R),
                                    mask_sb[:, slot, :].rearrange(
                                        "p (two q) -> p two q", two=2).bitcast(F32R))
                            for half in range(2):
                                kt = 2 * pr + half
                                src, coff = kt_src(kt)
                                vidx = src * 4 + (coff // KT)
                                first = (pr == 0 and half == 0)
                                last = (pr == npair - 1 and half == 1)
                                if not os.environ.get("WAN_NO_DEN"):
                                    nc.tensor.matmul(
                                        ps_den[:], ones_r[0:KT, 0:1].bitcast(F32R),
                                        expT[:, half, :].bitcast(F32R),
                                        start=first, stop=last)
                                nc.tensor.matmul(
                                    ps_o[:], v_h[:, vidx, :].bitcast(F32R),
                                    expT[:, half, :].bitcast(F32R),
                                    start=first, stop=last)
                        if os.environ.get("WAN_NO_RECIP"):
                            nc.vector.tensor_copy(
                                attnT[:, h, qb * CH:(qb + 1) * CH].bitcast(F32R),
                                ps_o[:])
                        else:
                            den_sb = att.tile([1, CH], F32, tag="densb")
                            nc.scalar.activation(den_sb[:], ps_den[:],
                                                 mybir.ActivationFunctionType.Copy)
                            rd = att.tile([1, CH], F32, tag="rd")
                            nc.vector.reciprocal(rd[:], den_sb[:])
                            nc.sync.dma_start(out=recip_scratch[h, qb, None, :],
                                              in_=rd[0:1, :])
                            rb = att.tile([128, CH], F32, tag="rb")
                            nc.sync.dma_start(
                                out=rb[:],
                                in_=recip_scratch[h, qb, None, :].partition_broadcast(128))
                            nc.vector.tensor_mul(
                                attnT[:, h, qb * CH:(qb + 1) * CH].bitcast(F32R),
                                ps_o[:], rb[:])

              # ---- phase 5: output projection ----
              with (
                  tc.tile_pool(name="op", bufs=3) as op,
                  tc.tile_pool(name="ps_op", bufs=2, space="PSUM") as ps_op,
              ):
                  for eb in range(NCB):
                      wt = op.tile([128, NKC, CB], F32, tag="wo")
                      nc.sync.dma_start(
                          out=wt[:].bitcast(F32R),
                          in_=Wo[:, eb * CB:(eb + 1) * CB]
                          .rearrange("(k p) c -> p k c", p=128).bitcast(F32R))
                      for st in range(NST):
                          acc = ps_op.tile([ST, CB], F32, tag=f"acc{st % 2}")
                          for ct in range(NKC):
                              nc.tensor.matmul(
                                  acc[:],
                                  attnT[:, ct, st * ST:(st + 1) * ST].bitcast(F32R),
                                  wt[:, ct, :].bitcast(F32R),
                                  start=(ct == 0), stop=False)
                          bo_t = op.tile([1, CB], F32, tag="bo")
                          nc.sync.dma_start(
                              out=bo_t[:].bitcast(F32R),
                              in_=bo[None, eb * CB:(eb + 1) * CB].bitcast(F32R))
                          nc.tensor.matmul(
                              acc[:], ones_r[0:1, 0:ST].bitcast(F32R),
                              bo_t[:].bitcast(F32R),
                              start=False, stop=True)
                          ot = op.tile([ST, CB], F32, tag="orow")
                          nc.scalar.activation(ot[:], acc[:],
                                               mybir.ActivationFunctionType.Copy)
                          nc.sync.dma_start(
                              out=out_own[st * ST:(st + 1) * ST,
                                          eb * CB:(eb + 1) * CB],
                              in_=ot[:])

    fix_waits(nc)
    return nc


# ---------------- host side ----------------

def _rope_cos_sin():
    """Static index maps for rope angle rows; returns function of freqs."""
    t_dim = ROT - 2 * (ROT // 3)   # 22
    s1 = ROT // 3                  # 21
    idx = np.arange(S)
    f_idx = idx // FRAME
    h_idx = (idx // W_GRID) % H_GRID
    w_idx = idx % W_GRID
    def build(freqs):
        ang = np.empty((S, ROT), np.float32)
        ang[:, :t_dim] = freqs[f_idx, :t_dim]
        ang[:, t_dim:t_dim + s1] = freqs[h_idx, t_dim:t_dim + s1]
        ang[:, t_dim + s1:] = freqs[w_idx, t_dim + s1:]
        cos = np.cos(ang).astype(np.float32)
        sin = np.sin(ang).astype(np.float32)
        cosf = np.repeat(cos, 2, axis=1)                     # [S, 128]
        sinf = np.empty((S, HD), np.float32)
        sinf[:, 0::2] = -sin
        sinf[:, 1::2] = sin
        return cosf, sinf
    return build

_build_cos_sin = _rope_cos_sin()


def _host_masks(c):
    """Mask tile pairs [N_MASKS, KT, 2*CH] for core c (1.0 allowed)."""
    frame_of = np.arange(S) // FRAME
    out = np.zeros((max(N_MASKS, 1), KT, 2 * CH), np.float32)
    slot = 0
    for qb in range(2):
        j = core_chunks(c)[qb]
        q0, q1 = chunk_rows(j)
        qf = frame_of[q0:q1]                                  # [252]
        for pr in masked_pairs(qb):
            krows = np.arange(2 * KT * pr, 2 * KT * pr + 2 * KT)
            kf = frame_of[krows]                              # [252]
            allow = (kf[:, None] <= qf[None, :])              # [252, 252]
            # mask layout [KT, 2, CH] flattened to [KT, 2*CH]
            m = np.zeros((KT, 2, CH), np.float32)
            m[:, 0, :] = allow[:KT, :]
            m[:, 1, :] = allow[KT:, :]
            out[slot] = m.reshape(KT, 2 * CH)
            slot += 1
    return out


_CACHE = {}


def _get_program():
    if "nc" not in _CACHE:
        _CACHE["nc"] = build_program()
    return _CACHE["nc"]


def kernel(**inputs):
    from concourse.bass_utils import run_bass_kernel_spmd

    x = np.asarray(inputs["x"], np.float32)       # [1, S, D]
    freqs = np.asarray(inputs["freqs"], np.float32)
    cosf, sinf = _build_cos_sin(freqs)

    common = {k: np.ascontiguousarray(np.asarray(inputs[k], np.float32))
              for k in ("Wq", "Wk", "Wv", "Wo", "bq", "bk", "bv", "bo",
                        "gq", "gk")}

    in_maps = []
    for c in range(NC):
        ja, jb = core_chunks(c)
        rows = np.concatenate([
            x[0, CH * ja:CH * (ja + 1)], x[0, CH * jb:CH * (jb + 1)]])
        cs = np.concatenate([
            cosf[CH * ja:CH * (ja + 1)], cosf[CH * jb:CH * (jb + 1)]])
        sn = np.concatenate([
            sinf[CH * ja:CH * (ja + 1)], sinf[CH * jb:CH * (jb + 1)]])
        m = {"x_own": np.ascontiguousarray(rows),
             "cosf": np.ascontiguousarray(cs),
             "sinf": np.ascontiguousarray(sn),
             "masks": _host_masks(c)}
        m.update(common)
        in_maps.append(m)

    nc = _get_program()
    res = run_bass_kernel_spmd(nc, in_maps, list(range(NC)))

    out = np.empty((1, S, D), np.float32)
    for c in range(NC):
        ja, jb = core_chunks(c)
        r = res.results[c]["out_own"]
        out[0, CH * ja:CH * (ja + 1)] = r[:CH]
        out[0, CH * jb:CH * (jb + 1)] = r[CH:]
    return out



# revision 10
# speedup vs baseline: 1.3396x; 1.3396x over previous
"""CausalWanSelfAttention on 8 Trainium2 NeuronCores.

Sharding: 16 query chunks of 252 rows; core c owns chunks (c, 15-c) for
causal load balance.  Each core projects q/k/v for its own 504 rows,
ropes+norms them, AllGathers K (transposed) and V (rows) across the 8
cores, runs frame-causal flash attention for its queries over all keys,
and projects the output rows.  Host assembles the full [1, 4032, 1536]
output from per-core row slices.
"""
import sys, os, types, math

sys.path.insert(0, "/opt/trn_rl_repo")

import numpy as np

import concourse.bass as bass
import concourse.tile as tile
from concourse import mybir
from concourse.masks import make_identity

F32 = mybir.dt.float32
F32R = mybir.dt.float32r
BF16 = mybir.dt.bfloat16

# Static problem geometry
S, D, H, HD = 4032, 1536, 12, 128
F_FRAMES, H_GRID, W_GRID = 9, 16, 28
FRAME = H_GRID * W_GRID            # 448
ROT = HD // 2                      # 64
EPS = 1e-6
SCALE = 1.0 / math.sqrt(HD)

NC = 8                             # cores
NCHUNK = 16                        # query chunks
CH = S // NCHUNK                   # 252 rows per chunk
KT = 126                           # k-tile rows (2 per chunk)
NKT = S // KT                      # 32 k-tiles
ST = 126                           # q row-tile (4 per core)
NST = 4
ROWS = 2 * CH                      # 504 own rows per core
CB = 512                           # projection column block
NCB = D // CB                      # 3
NKC = D // 128                     # 12 contraction chunks

# chunk pair owned by core c: (c, 15-c)
def core_chunks(c):
    return (c, NCHUNK - 1 - c)

# global row range of chunk j
def chunk_rows(j):
    return CH * j, CH * (j + 1)

# k-tile kt (global row order) -> (source core, col offset within its 504)
def kt_src(kt):
    j = kt // 2
    src = j if j < NC else NCHUNK - 1 - j
    sub = 0 if j < NC else 1
    return src, sub * CH + (kt % 2) * KT

# static per-qb k-loop bounds (tiles of KT), max over cores
def _bounds():
    bnd = []
    for qb in range(2):
        mx = 0
        for c in range(NC):
            j = core_chunks(c)[qb]
            last_row = CH * j + CH - 1
            keys = (last_row // FRAME + 1) * FRAME
            mx = max(mx, -(-keys // KT))
        bnd.append(mx)
    return bnd

BND = _bounds()                    # [18, 32]

# which k-tiles are all-allowed for every core (no mask needed)
def _clean_tiles():
    clean = []
    for qb in range(2):
        mn = S + 1
        for c in range(NC):
            j = core_chunks(c)[qb]
            first_row = CH * j
            keys = (first_row // FRAME + 1) * FRAME
            mn = min(mn, keys)
        clean.append(mn // KT)     # tiles [0, clean) need no mask
    return clean

CLEAN = _clean_tiles()             # [3, 17]

# masked tile-PAIRS per qb: pair p covers k-tiles (2p, 2p+1)
def masked_pairs(qb):
    lo, hi = CLEAN[qb], BND[qb]
    pairs = sorted({kt // 2 for kt in range(lo, hi)})
    return pairs

MP0, MP1 = masked_pairs(0), masked_pairs(1)
N_MASKS = len(MP0) + len(MP1)

DMA_HOIST_SKIP = {"InstCall", "InstNoOp"}


def fix_waits(nc):
    """This container's walrus rejects any instruction carrying more
    than one sync wait.  Hoist waits onto preceding NoOps (one wait per
    NoOp) on the same engine; sequencer program order preserves the
    semantics."""
    nop_id = [0]
    for f in nc.m.functions:
        for blk in f.blocks:
            out, changed = [], False
            for inst in blk.instructions:
                tname = type(inst).__name__
                si = inst.sync_info
                if (tname not in DMA_HOIST_SKIP and si is not None
                        and len(si.on_wait) > 1):
                    for w in list(si.on_wait):
                        nop = mybir.InstNoOp(name=f"I-waitnop-{nop_id[0]}")
                        nop_id[0] += 1
                        nop.engine = inst.engine
                        nop.sync_info = mybir.SyncInfo(on_wait=[w], on_update=[])
                        out.append(nop)
                    si.on_wait = []
                    changed = True
                out.append(inst)
            if changed:
                blk.instructions = out


def build_program():
    nc = bass.Bass(num_devices=NC)

    x_own = nc.dram_tensor("x_own", [ROWS, D], F32, kind="ExternalInput")
    Wq = nc.dram_tensor("Wq", [D, D], F32, kind="ExternalInput")
    Wk = nc.dram_tensor("Wk", [D, D], F32, kind="ExternalInput")
    Wv = nc.dram_tensor("Wv", [D, D], F32, kind="ExternalInput")
    Wo = nc.dram_tensor("Wo", [D, D], F32, kind="ExternalInput")
    bq = nc.dram_tensor("bq", [D], F32, kind="ExternalInput")
    bk = nc.dram_tensor("bk", [D], F32, kind="ExternalInput")
    bv = nc.dram_tensor("bv", [D], F32, kind="ExternalInput")
    bo = nc.dram_tensor("bo", [D], F32, kind="ExternalInput")
    gq = nc.dram_tensor("gq", [D], F32, kind="ExternalInput")
    gk = nc.dram_tensor("gk", [D], F32, kind="ExternalInput")
    cosf = nc.dram_tensor("cosf", [ROWS, HD], F32, kind="ExternalInput")
    sinf = nc.dram_tensor("sinf", [ROWS, HD], F32, kind="ExternalInput")
    masks = nc.dram_tensor("masks", [max(N_MASKS, 1), KT, 2 * CH], BF16,
                           kind="ExternalInput")
    out_own = nc.dram_tensor("out_own", [ROWS, D], F32, kind="ExternalOutput")

    kT_bounce = nc.dram_tensor("kT_bounce", [D, ROWS], BF16)
    v_bounce = nc.dram_tensor("v_bounce", [ROWS, D], BF16)
    kT_g = nc.dram_tensor("kT_g", [NC, D, ROWS], BF16, addr_space="Shared")
    v_g = nc.dram_tensor("v_g", [NC, ROWS, D], BF16, addr_space="Shared")
    recip_scratch = nc.dram_tensor("recip_scratch", [H, 2, CH], F32)

    with tile.TileContext(nc) as tc:
        with tc.tile_pool(name="persist", bufs=1) as persist:
            # ---- constants ----
            ident = persist.tile([128, 128], F32)
            make_identity(nc, ident[:])
            ones_f = persist.tile([128, 128], F32)
            nc.gpsimd.memset(ones_f[:], 1.0)
            ones_r = persist.tile([128, 128], F32)
            nc.vector.tensor_copy(ones_r[:].bitcast(F32R), ones_f[:])
            ones_bf = persist.tile([128, 1], BF16)
            nc.gpsimd.memset(ones_bf[:], 1.0)
            eps_c = persist.tile([128, 1], F32)
            nc.gpsimd.memset(eps_c[:], EPS)
            rcpD_c = persist.tile([128, 1], F32)
            nc.gpsimd.memset(rcpD_c[:], 1.0 / D)
            scl_c = persist.tile([128, 1], F32)
            nc.gpsimd.memset(scl_c[:], SCALE)



            qT = persist.tile([128, H, ROWS], BF16)      # q transposed (bf16)

            # ---- phase 1: x rows -> xT ----
            with (
                tc.tile_pool(name="p1", bufs=2) as p1,
                tc.tile_pool(name="xTp", bufs=1) as xTp,
                tc.tile_pool(name="gpool", bufs=1) as gpool,
                tc.tile_pool(name="bpool", bufs=2) as bpool,
                tc.tile_pool(name="ps_tp", bufs=2, space="PSUM") as ps_tp,
                tc.tile_pool(name="ps_acc", bufs=1, space="PSUM") as ps_acc,
            ):
                cs_sb = gpool.tile([ST, 2, NST, HD], F32, tag="cs")
                nc.sync.dma_start(out=cs_sb[:, 0],
                                  in_=cosf.rearrange("(t p) f -> p t f", p=ST))
                nc.sync.dma_start(out=cs_sb[:, 1],
                                  in_=sinf.rearrange("(t p) f -> p t f", p=ST))
                gq_b = gpool.tile([128, D], F32, tag="gq")
                nc.sync.dma_start(out=gq_b[:], in_=gq[None, :].partition_broadcast(128))
                gk_b = gpool.tile([128, D], F32, tag="gk")
                nc.sync.dma_start(out=gk_b[:], in_=gk[None, :].partition_broadcast(128))

                def bias_slice(b, cb):
                    t = bpool.tile([1, CB], F32, tag="bias")
                    nc.sync.dma_start(
                        out=t[:].bitcast(F32R),
                        in_=b[None, cb * CB:(cb + 1) * CB].bitcast(F32R))
                    return t

                xT = xTp.tile([128, NKC, ROWS], F32)     # f32r content
                for st in range(NST):
                    xr = p1.tile([ST, D], F32, tag="xrow")
                    nc.sync.dma_start(out=xr[:], in_=x_own[st * ST:(st + 1) * ST, :])
                    for kc in range(NKC):
                        pt = ps_tp.tile([128, ST], F32, tag="tp")
                        nc.tensor.transpose(pt[:], xr[:, kc * 128:(kc + 1) * 128], ident[0:ST, 0:ST])
                        nc.vector.tensor_copy(
                            xT[:, kc, st * ST:(st + 1) * ST].bitcast(F32R), pt[:])

                # ---- phase 2: projections ----
                def project(W, bias_idx, out_cb):
                    """h = x @ W + b per s-tile/c-block; out_cb(st, cb, acc)
                    consumes the PSUM tile."""
                    with tc.tile_pool(name="wpool", bufs=2) as wp:
                        for cb in range(NCB):
                            wt = wp.tile([128, NKC, CB], F32, tag="w")
                            nc.sync.dma_start(
                                out=wt[:].bitcast(F32R),
                                in_=W[:, cb * CB:(cb + 1) * CB]
                                .rearrange("(k p) c -> p k c", p=128).bitcast(F32R))
                            for st in range(NST):
                                acc = ps_acc.tile([ST, CB], F32, tag=f"acc{st % 2}")
                                for kc in range(NKC):
                                    nc.tensor.matmul(
                                        acc[:],
                                        xT[:, kc, st * ST:(st + 1) * ST].bitcast(F32R),
                                        wt[:, kc, :].bitcast(F32R),
                                        start=(kc == 0), stop=False)
                                nc.tensor.matmul(
                                    acc[:], ones_r[0:1, 0:ST].bitcast(F32R),
                                    bias_slice(bias_idx, cb)[:].bitcast(F32R),
                                    start=False, stop=True)
                                out_cb(st, cb, acc)

                # --- v: no norm; straight to bounce (bf16) ---
                def v_out(st, cb, acc):
                    vr = p1.tile([ST, CB], BF16, tag="vrow")
                    nc.scalar.activation(vr[:], acc[:],
                                         mybir.ActivationFunctionType.Copy)
                    nc.sync.dma_start(
                        out=v_bounce[st * ST:(st + 1) * ST, cb * CB:(cb + 1) * CB],
                        in_=vr[:])

                # --- q/k: rmsnorm + gain + rope, then transpose ---
                def norm_rope(W, bias, g_b, sink):
                    """sink(kc, st, tile128x126_f32r_psum) consumes transposed
                    normalized+roped tiles."""
                    with (
                        tc.tile_pool(name="nr", bufs=2) as nr,
                        tc.tile_pool(name="nr1", bufs=1) as nr1,
                    ):
                        rows_all = nr1.tile([ST, NST, D], F32)
                        var_p = nr1.tile([ST, NST, NCB], F32)
                        # pass A: h = x@W + b -> rows_all; sumsq -> var_p
                        for cb in range(NCB):
                            wt = nr.tile([128, NKC, CB], F32, tag="w2")
                            nc.sync.dma_start(
                                out=wt[:].bitcast(F32R),
                                in_=W[:, cb * CB:(cb + 1) * CB]
                                .rearrange("(k p) c -> p k c", p=128).bitcast(F32R))
                            for st in range(NST):
                                acc = ps_acc.tile([ST, CB], F32, tag=f"acc{st % 2}")
                                for kc in range(NKC):
                                    nc.tensor.matmul(
                                        acc[:],
                                        xT[:, kc, st * ST:(st + 1) * ST].bitcast(F32R),
                                        wt[:, kc, :].bitcast(F32R),
                                        start=(kc == 0), stop=False)
                                nc.tensor.matmul(
                                    acc[:], ones_r[0:1, 0:ST].bitcast(F32R),
                                    bias_slice(bias, cb)[:].bitcast(F32R),
                                    start=False, stop=True)
                                sq = nr.tile([ST, CB], F32, tag="sq")
                                nc.scalar.activation(
                                    sq[:], acc[:],
                                    mybir.ActivationFunctionType.Square,
                                    accum_out=var_p[:, st, cb:cb + 1])
                                nc.vector.tensor_copy(
                                    rows_all[:, st, cb * CB:(cb + 1) * CB], acc[:])
                        # pass B: rstd, normalize, gain, rope, transpose
                        for st in range(NST):
                            var = nr.tile([ST, 1], F32, tag="var")
                            nc.vector.reduce_sum(var[:], var_p[:, st, :],
                                                 axis=mybir.AxisListType.X)
                            sdt = nr.tile([ST, 1], F32, tag="sd")
                            nc.scalar.activation(
                                sdt[:], var[:], mybir.ActivationFunctionType.Sqrt,
                                bias=eps_c[0:ST, :], scale=rcpD_c[0:ST, :])
                            rstd = nr.tile([ST, 1], F32, tag="rstd")
                            nc.vector.reciprocal(rstd[:], sdt[:])
                            rows = nr.tile([ST, D], F32, tag="rows")
                            nc.scalar.activation(
                                rows[:], rows_all[:, st, :],
                                mybir.ActivationFunctionType.Copy,
                                scale=rstd[:])
                            nc.vector.tensor_mul(rows[:], rows[:], g_b[0:ST, :])
                            # rope: out = rows*cosF + swap(rows)*sinF
                            swap = nr.tile([ST, D], F32, tag="swap")
                            r3 = rows[:].rearrange("p (h r two) -> p h r two",
                                                   two=2, r=ROT)
                            s3 = swap[:].rearrange("p (h r two) -> p h r two",
                                                   two=2, r=ROT)
                            nc.vector.tensor_copy(s3[:, :, :, 0], r3[:, :, :, 1])
                            nc.vector.tensor_copy(s3[:, :, :, 1], r3[:, :, :, 0])
                            cosb = (cs_sb[:, 0, st, :].unsqueeze(1)
                                    .broadcast_to((ST, H, HD)))
                            sinb = (cs_sb[:, 1, st, :].unsqueeze(1)
                                    .broadcast_to((ST, H, HD)))
                            rr = rows[:].rearrange("p (h f) -> p h f", h=H)
                            sr = swap[:].rearrange("p (h f) -> p h f", h=H)
                            nc.vector.tensor_mul(rr[:], rr[:], cosb)
                            nc.vector.tensor_mul(sr[:], sr[:], sinb)
                            nc.vector.tensor_add(rows[:], rows[:], swap[:])
                            for kc in range(NKC):
                                pt = ps_tp.tile([128, ST], F32, tag="tp")
                                nc.tensor.transpose(
                                    pt[:], rows[:, kc * 128:(kc + 1) * 128],
                                    ident[0:ST, 0:ST])
                                sink(kc, st, pt)

                def q_sink(kc, st, pt):
                    nc.vector.tensor_copy(
                        qT[:, kc, st * ST:(st + 1) * ST], pt[:])

                ktmp_pool = tc.tile_pool(name="ktmp", bufs=3)
                ktmp = ktmp_pool.__enter__()

                kstage = {}

                def k_sink(kc, st, pt):
                    if st not in kstage:
                        kstage[st] = ktmp.tile([128, NKC, ST], BF16, tag="ktr", name=f"kstage{st}")
                    kt_sb = kstage[st]
                    nc.vector.tensor_copy(kt_sb[:, kc, :], pt[:])
                    if kc == NKC - 1:
                        nc.sync.dma_start(
                            out=kT_bounce[:, st * ST:(st + 1) * ST]
                            .rearrange("(k p) s -> p k s", p=128),
                            in_=kt_sb[:])
                        del kstage[st]

                # k first: its gather is on the critical path to attention
                norm_rope(Wk, bk, gk_b, k_sink)
                nc.gpsimd.collective_compute(
                    "AllGather", mybir.AluOpType.bypass,
                    replica_groups=[list(range(NC))],
                    ins=[kT_bounce[:].opt()], outs=[kT_g[:].opt()])
                project(Wv, bv, v_out)
                nc.gpsimd.collective_compute(
                    "AllGather", mybir.AluOpType.bypass,
                    replica_groups=[list(range(NC))],
                    ins=[v_bounce[:].opt()], outs=[v_g[:].opt()])
                norm_rope(Wq, bq, gq_b, q_sink)
                ktmp_pool.__exit__(None, None, None)

            # ---- phase 4: attention ----
            with tc.tile_pool(name="late", bufs=1) as late:
              attnT = late.tile([128, H, ROWS], F32)   # attention out^T, f32r
              with (
                tc.tile_pool(name="att", bufs=2) as att,
                tc.tile_pool(name="mk", bufs=1) as mk,
                tc.tile_pool(name="psO", bufs=2, space="PSUM") as psO,
                tc.tile_pool(name="ps_s", bufs=2, space="PSUM") as ps_s_pool,
            ):
                mask_sb = mk.tile([KT, max(N_MASKS, 1), 2 * CH], BF16)
                nc.sync.dma_start(out=mask_sb[:],
                                  in_=masks.rearrange("n p q -> p n q"))
                # mask index lookup: (qb, pair) -> slot
                mask_slot = {}
                for p in MP0:
                    mask_slot[(0, p)] = len(mask_slot)
                for p in MP1:
                    mask_slot[(1, p)] = len(mask_slot)

                for h in range(int(os.environ.get('WAN_ATT_H', str(H)))):
                    kT_h = att.tile([128, NC, ROWS], BF16, tag="kTh")
                    nc.sync.dma_start(
                        out=kT_h[:],
                        in_=kT_g[:, h * HD:(h + 1) * HD, :]
                        .rearrange("s p c -> p s c"))
                    v_h = att.tile([KT, NC * 4, HD], BF16, tag="vh")
                    for src in range(NC):
                        nc.sync.dma_start(
                            out=v_h[:, src * 4:(src + 1) * 4, :],
                            in_=v_g[src, :, h * HD:(h + 1) * HD]
                            .rearrange("(r p) d -> p r d", p=KT))
                    for qb in range(2):
                        ps_o = psO.tile([128, CH], F32, tag="o")
                        ps_den = (None if os.environ.get("WAN_NO_DEN")
                                  else psO.tile([1, CH], F32, tag="den"))
                        npair = BND[qb] // 2
                        for pr in range(npair):
                            ps_s = ps_s_pool.tile([KT, 2, CH], F32, tag="s")
                            for half in range(2):
                                kt = 2 * pr + half
                                src, coff = kt_src(kt)
                                nc.tensor.matmul(
                                    ps_s[:, half, :],
                                    kT_h[:, src, coff:coff + KT],
                                    qT[:, h, qb * CH:(qb + 1) * CH],
                                    start=True, stop=True)
                            expT = att.tile([KT, 2, CH], BF16, tag="expT")
                            nc.scalar.activation(
                                expT[:], ps_s[:],
                                mybir.ActivationFunctionType.Exp,
                                scale=scl_c[0:KT, :])
                            slot = mask_slot.get((qb, pr))
                            if slot is not None:
                                nc.vector.tensor_mul(
                                    expT[:], expT[:],
                                    mask_sb[:, slot, :].rearrange(
                                        "p (two q) -> p two q", two=2))
                            for half in range(2):
                                kt = 2 * pr + half
                                src, coff = kt_src(kt)
                                vidx = src * 4 + (coff // KT)
                                first = (pr == 0 and half == 0)
                                last = (pr == npair - 1 and half == 1)
                                if not os.environ.get("WAN_NO_DEN"):
                                    nc.tensor.matmul(
                                        ps_den[:], ones_bf[0:KT, 0:1],
                                        expT[:, half, :],
                                        start=first, stop=last)
                                nc.tensor.matmul(
                                    ps_o[:], v_h[:, vidx, :],
                                    expT[:, half, :],
                                    start=first, stop=last)
                        if os.environ.get("WAN_NO_RECIP"):
                            nc.vector.tensor_copy(
                                attnT[:, h, qb * CH:(qb + 1) * CH].bitcast(F32R),
                                ps_o[:])
                        else:
                            den_sb = att.tile([1, CH], F32, tag="densb")
                            nc.scalar.activation(den_sb[:], ps_den[:],
                                                 mybir.ActivationFunctionType.Copy)
                            rd = att.tile([1, CH], F32, tag="rd")
                            nc.vector.reciprocal(rd[:], den_sb[:])
                            nc.sync.dma_start(out=recip_scratch[h, qb, None, :],
                                              in_=rd[0:1, :])
                            rb = att.tile([128, CH], F32, tag="rb")
                            nc.sync.dma_start(
                                out=rb[:],
                                in_=recip_scratch[h, qb, None, :].partition_broadcast(128))
                            nc.vector.tensor_mul(
                                attnT[:, h, qb * CH:(qb + 1) * CH].bitcast(F32R),
                                ps_o[:], rb[:])

              # ---- phase 5: output projection ----
              with (
                  tc.tile_pool(name="op", bufs=3) as op,
                  tc.tile_pool(name="ps_op", bufs=2, space="PSUM") as ps_op,
              ):
                  for eb in range(NCB):
                      wt = op.tile([128, NKC, CB], F32, tag="wo")
                      nc.sync.dma_start(
                          out=wt[:].bitcast(F32R),
                          in_=Wo[:, eb * CB:(eb + 1) * CB]
                          .rearrange("(k p) c -> p k c", p=128).bitcast(F32R))
                      for st in range(NST):
                          acc = ps_op.tile([ST, CB], F32, tag=f"acc{st % 2}")
                          for ct in range(NKC):
                              nc.tensor.matmul(
                                  acc[:],
                                  attnT[:, ct, st * ST:(st + 1) * ST].bitcast(F32R),
                                  wt[:, ct, :].bitcast(F32R),
                                  start=(ct == 0), stop=False)
                          bo_t = op.tile([1, CB], F32, tag="bo")
                          nc.sync.dma_start(
                              out=bo_t[:].bitcast(F32R),
                              in_=bo[None, eb * CB:(eb + 1) * CB].bitcast(F32R))
                          nc.tensor.matmul(
                              acc[:], ones_r[0:1, 0:ST].bitcast(F32R),
                              bo_t[:].bitcast(F32R),
                              start=False, stop=True)
                          ot = op.tile([ST, CB], F32, tag="orow")
                          nc.scalar.activation(ot[:], acc[:],
                                               mybir.ActivationFunctionType.Copy)
                          nc.sync.dma_start(
                              out=out_own[st * ST:(st + 1) * ST,
                                          eb * CB:(eb + 1) * CB],
                              in_=ot[:])

    fix_waits(nc)
    return nc


# ---------------- host side ----------------

def _rope_cos_sin():
    """Static index maps for rope angle rows; returns function of freqs."""
    t_dim = ROT - 2 * (ROT // 3)   # 22
    s1 = ROT // 3                  # 21
    idx = np.arange(S)
    f_idx = idx // FRAME
    h_idx = (idx // W_GRID) % H_GRID
    w_idx = idx % W_GRID
    def build(freqs):
        ang = np.empty((S, ROT), np.float32)
        ang[:, :t_dim] = freqs[f_idx, :t_dim]
        ang[:, t_dim:t_dim + s1] = freqs[h_idx, t_dim:t_dim + s1]
        ang[:, t_dim + s1:] = freqs[w_idx, t_dim + s1:]
        cos = np.cos(ang).astype(np.float32)
        sin = np.sin(ang).astype(np.float32)
        cosf = np.repeat(cos, 2, axis=1)                     # [S, 128]
        sinf = np.empty((S, HD), np.float32)
        sinf[:, 0::2] = -sin
        sinf[:, 1::2] = sin
        return cosf, sinf
    return build

_build_cos_sin = _rope_cos_sin()


def _host_masks(c):
    """Mask tile pairs [N_MASKS, KT, 2*CH] for core c (1.0 allowed)."""
    frame_of = np.arange(S) // FRAME
    out = np.zeros((max(N_MASKS, 1), KT, 2 * CH), np.float32)
    slot = 0
    for qb in range(2):
        j = core_chunks(c)[qb]
        q0, q1 = chunk_rows(j)
        qf = frame_of[q0:q1]                                  # [252]
        for pr in masked_pairs(qb):
            krows = np.arange(2 * KT * pr, 2 * KT * pr + 2 * KT)
            kf = frame_of[krows]                              # [252]
            allow = (kf[:, None] <= qf[None, :])              # [252, 252]
            # mask layout [KT, 2, CH] flattened to [KT, 2*CH]
            m = np.zeros((KT, 2, CH), np.float32)
            m[:, 0, :] = allow[:KT, :]
            m[:, 1, :] = allow[KT:, :]
            out[slot] = m.reshape(KT, 2 * CH)
            slot += 1
    import ml_dtypes
    return out.astype(ml_dtypes.bfloat16)


_CACHE = {}


def _get_program():
    if "nc" not in _CACHE:
        _CACHE["nc"] = build_program()
    return _CACHE["nc"]


def kernel(**inputs):
    from concourse.bass_utils import run_bass_kernel_spmd

    x = np.asarray(inputs["x"], np.float32)       # [1, S, D]
    freqs = np.asarray(inputs["freqs"], np.float32)
    cosf, sinf = _build_cos_sin(freqs)

    common = {k: np.ascontiguousarray(np.asarray(inputs[k], np.float32))
              for k in ("Wq", "Wk", "Wv", "Wo", "bq", "bk", "bv", "bo",
                        "gq", "gk")}

    in_maps = []
    for c in range(NC):
        ja, jb = core_chunks(c)
        rows = np.concatenate([
            x[0, CH * ja:CH * (ja + 1)], x[0, CH * jb:CH * (jb + 1)]])
        cs = np.concatenate([
            cosf[CH * ja:CH * (ja + 1)], cosf[CH * jb:CH * (jb + 1)]])
        sn = np.concatenate([
            sinf[CH * ja:CH * (ja + 1)], sinf[CH * jb:CH * (jb + 1)]])
        m = {"x_own": np.ascontiguousarray(rows),
             "cosf": np.ascontiguousarray(cs),
             "sinf": np.ascontiguousarray(sn),
             "masks": _host_masks(c)}
        m.update(common)
        in_maps.append(m)

    nc = _get_program()
    res = run_bass_kernel_spmd(nc, in_maps, list(range(NC)))

    out = np.empty((1, S, D), np.float32)
    for c in range(NC):
        ja, jb = core_chunks(c)
        r = res.results[c]["out_own"]
        out[0, CH * ja:CH * (ja + 1)] = r[:CH]
        out[0, CH * jb:CH * (jb + 1)] = r[CH:]
    return out



# revision 30
# speedup vs baseline: 1.3460x; 1.0048x over previous
"""CausalWanSelfAttention on 8 Trainium2 NeuronCores.

Sharding: 16 query chunks of 252 rows; core c owns chunks (c, 15-c) for
causal load balance.  Each core projects q/k/v for its own 504 rows,
ropes+norms them (k/q), AllGathers K^T and V (bf16, split into two
head-group collectives each so attention can start early), runs
frame-causal attention for its queries over all keys with the two query
chunks fused into shared k-tile matmuls (N=504), and projects the
output rows.  Host assembles the full [1, 4032, 1536] output.
"""
import sys, os, math

sys.path.insert(0, "/opt/trn_rl_repo")

import numpy as np

import concourse.bass as bass
import concourse.tile as tile
from concourse import mybir
from concourse.masks import make_identity

F32 = mybir.dt.float32
F32R = mybir.dt.float32r
BF16 = mybir.dt.bfloat16

# Static problem geometry
S, D, H, HD = 4032, 1536, 12, 128
F_FRAMES, H_GRID, W_GRID = 9, 16, 28
FRAME = H_GRID * W_GRID            # 448
ROT = HD // 2                      # 64
EPS = 1e-6
SCALE = 1.0 / math.sqrt(HD)

NC = 8                             # cores
NCHUNK = 16                        # query chunks
CH = S // NCHUNK                   # 252 rows per chunk
KT = 126                           # k-tile rows
NKT = S // KT                      # 32 k-tiles
ST = 126                           # row-tile (4 per core)
NST = 4
ROWS = 2 * CH                      # 504 own rows per core
CB = 512                           # projection column block (q/k/o)
NCB = D // CB                      # 3
CBV = 384                          # v projection column block
NCBV = D // CBV                    # 4
NKC = D // 128                     # 12 contraction chunks
DH2 = D // 2                       # 768: head-group half (heads 0-5 / 6-11)
HG = H // 2                        # 6 heads per group

# chunk pair owned by core c: (c, 15-c)
def core_chunks(c):
    return (c, NCHUNK - 1 - c)

# global row range of chunk j
def chunk_rows(j):
    return CH * j, CH * (j + 1)

# k-tile kt (global row order) -> (source core, col offset within its 504)
def kt_src(kt):
    j = kt // 2
    src = j if j < NC else NCHUNK - 1 - j
    sub = 0 if j < NC else 1
    return src, sub * CH + (kt % 2) * KT

# static per-qb k-loop bounds (tiles of KT), max over cores
def _bounds():
    bnd = []
    for qb in range(2):
        mx = 0
        for c in range(NC):
            j = core_chunks(c)[qb]
            last_row = CH * j + CH - 1
            keys = (last_row // FRAME + 1) * FRAME
            mx = max(mx, -(-keys // KT))
        bnd.append(mx)
    return bnd

BND = _bounds()                    # [18, 32]

# k-tiles that are all-allowed for every core (no mask needed)
def _clean_tiles():
    clean = []
    for qb in range(2):
        mn = S + 1
        for c in range(NC):
            j = core_chunks(c)[qb]
            first_row = CH * j
            keys = (first_row // FRAME + 1) * FRAME
            mn = min(mn, keys)
        clean.append(mn // KT)     # tiles [0, clean) need no mask
    return clean

CLEAN = _clean_tiles()             # [3, 17]

# fused region: kt < BND[0] covers both query chunks (N = 504);
# solo region: BND[0] <= kt < BND[1] covers only chunk 1 (N = 252)
FUSED_MASK_KTS = list(range(CLEAN[0], BND[0]))          # need any mask
SOLO_MASK_KTS = list(range(max(CLEAN[1], BND[0]), BND[1]))
N_MASKS = len(FUSED_MASK_KTS) + len(SOLO_MASK_KTS)

DMA_HOIST_SKIP = {"InstCall", "InstNoOp"}


def fix_waits(nc):
    """This container's walrus rejects any instruction carrying more
    than one sync wait.  Hoist waits onto preceding NoOps (one wait per
    NoOp) on the same engine; sequencer program order preserves the
    semantics."""
    nop_id = [0]
    for f in nc.m.functions:
        for blk in f.blocks:
            out, changed = [], False
            for inst in blk.instructions:
                tname = type(inst).__name__
                si = inst.sync_info
                if (tname not in DMA_HOIST_SKIP and si is not None
                        and len(si.on_wait) > 1):
                    for w in list(si.on_wait):
                        nop = mybir.InstNoOp(name=f"I-waitnop-{nop_id[0]}")
                        nop_id[0] += 1
                        nop.engine = inst.engine
                        nop.sync_info = mybir.SyncInfo(on_wait=[w], on_update=[])
                        out.append(nop)
                    si.on_wait = []
                    changed = True
                out.append(inst)
            if changed:
                blk.instructions = out


def build_program():
    nc = bass.Bass(num_devices=NC)

    x_own = nc.dram_tensor("x_own", [ROWS, D], F32, kind="ExternalInput")
    Wq = nc.dram_tensor("Wq", [D, D], BF16, kind="ExternalInput")
    Wk = nc.dram_tensor("Wk", [D, D], BF16, kind="ExternalInput")
    Wv = nc.dram_tensor("Wv", [D, D], BF16, kind="ExternalInput")
    Wo = nc.dram_tensor("Wo", [D, D], BF16, kind="ExternalInput")
    bq = nc.dram_tensor("bq", [D], F32, kind="ExternalInput")
    bk = nc.dram_tensor("bk", [D], F32, kind="ExternalInput")
    bv = nc.dram_tensor("bv", [D], F32, kind="ExternalInput")
    bo = nc.dram_tensor("bo", [D], F32, kind="ExternalInput")
    gq = nc.dram_tensor("gq", [D], F32, kind="ExternalInput")
    gk = nc.dram_tensor("gk", [D], F32, kind="ExternalInput")
    cosf = nc.dram_tensor("cosf", [ROWS, HD], F32, kind="ExternalInput")
    sinf = nc.dram_tensor("sinf", [ROWS, HD], F32, kind="ExternalInput")
    masks = nc.dram_tensor("masks", [max(N_MASKS, 1), KT, 2 * CH], BF16,
                           kind="ExternalInput")
    out_own = nc.dram_tensor("out_own", [ROWS, D], F32, kind="ExternalOutput")

    kT_bounce = nc.dram_tensor("kT_bounce", [D, ROWS], BF16)
    q_bounce = nc.dram_tensor("q_bounce", [D, ROWS], BF16)
    v_bounce = nc.dram_tensor("v_bounce", [2, ROWS, DH2], BF16)
    kT_g = [nc.dram_tensor(f"kT_g{g}", [NC, DH2, ROWS], BF16,
                           addr_space="Shared") for g in range(2)]
    v_g = [nc.dram_tensor(f"v_g{g}", [NC, ROWS, DH2], BF16,
                          addr_space="Shared") for g in range(2)]
    recip_scratch = nc.dram_tensor("recip_scratch", [H, 2, CH], F32)

    DEBUG = bool(os.environ.get("WAN_DEBUG"))
    if DEBUG:
        dbg_qT = nc.dram_tensor("dbg_qT", [128, H, ROWS], BF16,
                                kind="ExternalOutput")
        dbg_kg = [nc.dram_tensor(f"dbg_kg{g}", [NC, DH2, ROWS], BF16,
                                 kind="ExternalOutput") for g in range(2)]
        dbg_vg = [nc.dram_tensor(f"dbg_vg{g}", [NC, ROWS, DH2], BF16,
                                 kind="ExternalOutput") for g in range(2)]
        dbg_attnT = nc.dram_tensor("dbg_attnT", [128, H, ROWS], BF16,
                                   kind="ExternalOutput")
        dbg_recip = nc.dram_tensor("dbg_recip", [H, 2, CH], F32,
                                   kind="ExternalOutput")
        dbg_qT2 = nc.dram_tensor("dbg_qT2", [128, H, ROWS], BF16,
                                 kind="ExternalOutput")
        dbg_mask = nc.dram_tensor("dbg_mask", [KT, max(N_MASKS, 1), 2 * CH],
                                  BF16, kind="ExternalOutput")
        dbg_gq = nc.dram_tensor("dbg_gq", [128, D], F32, kind="ExternalOutput")
        dbg_gk = nc.dram_tensor("dbg_gk", [128, D], F32, kind="ExternalOutput")
        dbg_qb = nc.dram_tensor("dbg_qb", [D, ROWS], BF16,
                                kind="ExternalOutput")

    with tile.TileContext(nc) as tc:
        with tc.tile_pool(name="persist", bufs=1) as persist:
            # ---- constants ----
            ident = persist.tile([128, 128], F32)
            make_identity(nc, ident[:])
            ones_f = persist.tile([128, 128], F32)
            nc.gpsimd.memset(ones_f[:], 1.0)
            ones_r = persist.tile([128, 128], F32)
            nc.vector.tensor_copy(ones_r[:].bitcast(F32R), ones_f[:])
            ones_bf = persist.tile([128, 128], BF16)
            nc.gpsimd.memset(ones_bf[:], 1.0)
            eps_c = persist.tile([128, 1], F32)
            nc.gpsimd.memset(eps_c[:], EPS)
            rcpD_c = persist.tile([128, 1], F32)
            nc.gpsimd.memset(rcpD_c[:], 1.0 / D)
            scl_c = persist.tile([128, 1], F32)
            nc.gpsimd.memset(scl_c[:], SCALE)

            # ---- phase 1: x rows -> xT ----
            with (
                tc.tile_pool(name="p1", bufs=2) as p1,
                tc.tile_pool(name="xTp", bufs=1) as xTp,
                tc.tile_pool(name="gpool", bufs=1) as gpool,
                tc.tile_pool(name="bpool", bufs=2) as bpool,
                tc.tile_pool(name="ps_tp", bufs=2, space="PSUM") as ps_tp,
                tc.tile_pool(name="ps_acc", bufs=1, space="PSUM") as ps_acc,
            ):
                cs_sb = gpool.tile([ST, 2, NST, HD], F32, tag="cs")
                nc.sync.dma_start(out=cs_sb[:, 0],
                                  in_=cosf.rearrange("(t p) f -> p t f", p=ST))
                nc.sync.dma_start(out=cs_sb[:, 1],
                                  in_=sinf.rearrange("(t p) f -> p t f", p=ST))
                gq_b = gpool.tile([128, D], F32, tag="gq")
                nc.sync.dma_start(out=gq_b[:], in_=gq[None, :].partition_broadcast(128))
                gk_b = gpool.tile([128, D], F32, tag="gk")
                nc.sync.dma_start(out=gk_b[:], in_=gk[None, :].partition_broadcast(128))

                def bias_slice(b, cb, cbs):
                    t = bpool.tile([1, cbs], F32, tag="bias")
                    nc.sync.dma_start(
                        out=t[:].bitcast(F32R),
                        in_=b[None, cb * cbs:(cb + 1) * cbs].bitcast(F32R))
                    t16 = bpool.tile([1, cbs], BF16, tag="bias16")
                    nc.scalar.activation(t16[:], t[:],
                                         mybir.ActivationFunctionType.Copy)
                    return t16

                xT = xTp.tile([128, NKC, ROWS], BF16)
                for st in range(NST):
                    xr = p1.tile([ST, D], F32, tag="xrow")
                    nc.sync.dma_start(out=xr[:], in_=x_own[st * ST:(st + 1) * ST, :])
                    for kc in range(NKC):
                        pt = ps_tp.tile([128, ST], F32, tag="tp")
                        nc.tensor.transpose(pt[:], xr[:, kc * 128:(kc + 1) * 128], ident[0:ST, 0:ST])
                        nc.vector.tensor_copy(
                            xT[:, kc, st * ST:(st + 1) * ST], pt[:])

                # ---- phase 2: projections ----
                def project(W, bias_idx, out_cb, cbs, ncbs):
                    """h = x @ W + b per s-tile/c-block; out_cb(st, cb, acc)
                    consumes the PSUM tile."""
                    with tc.tile_pool(name="wpool", bufs=2) as wp:
                        for cb in range(ncbs):
                            wt = wp.tile([128, NKC, cbs], BF16, tag="w")
                            nc.sync.dma_start(
                                out=wt[:],
                                in_=W[:, cb * cbs:(cb + 1) * cbs]
                                .rearrange("(k p) c -> p k c", p=128))
                            for st in range(NST):
                                acc = ps_acc.tile([ST, cbs], F32, tag=f"acc{st % 2}")
                                for kc in range(NKC):
                                    nc.tensor.matmul(
                                        acc[:],
                                        xT[:, kc, st * ST:(st + 1) * ST],
                                        wt[:, kc, :],
                                        start=(kc == 0), stop=False)
                                nc.tensor.matmul(
                                    acc[:], ones_bf[0:1, 0:ST],
                                    bias_slice(bias_idx, cb, cbs)[:],
                                    start=False, stop=True)
                                out_cb(st, cb, acc)

                # --- v: no norm; straight to bounce (bf16, head-group split) ---
                def v_out(st, cb, acc):
                    vr = p1.tile([ST, CBV], BF16, tag="vrow")
                    nc.scalar.activation(vr[:], acc[:],
                                         mybir.ActivationFunctionType.Copy)
                    nc.sync.dma_start(
                        out=v_bounce[cb // 2, st * ST:(st + 1) * ST,
                                     (cb % 2) * CBV:(cb % 2 + 1) * CBV],
                        in_=vr[:])

                # --- q/k: rmsnorm + gain + rope, then transpose ---
                def norm_rope(W, bias, g_b, sink, mid=None, kc_outer=False):
                    """sink(kc, st, tile) consumes transposed normalized+roped
                    tiles.  mid() runs between pass A and pass B (fills the
                    PE while pass B's vector chain runs)."""
                    with (
                        tc.tile_pool(name="nr", bufs=2) as nr,
                        tc.tile_pool(name="nr1", bufs=1) as nr1,
                    ):
                        rows_all = nr1.tile([ST, NST, D], F32)
                        var_p = nr1.tile([ST, NST, NCB], F32)
                        # pass A: h = x@W + b -> rows_all; sumsq -> var_p
                        for cb in range(NCB):
                            wt = nr.tile([128, NKC, CB], BF16, tag="w2")
                            nc.sync.dma_start(
                                out=wt[:],
                                in_=W[:, cb * CB:(cb + 1) * CB]
                                .rearrange("(k p) c -> p k c", p=128))
                            for st in range(NST):
                                acc = ps_acc.tile([ST, CB], F32, tag=f"acc{st % 2}")
                                for kc in range(NKC):
                                    nc.tensor.matmul(
                                        acc[:],
                                        xT[:, kc, st * ST:(st + 1) * ST],
                                        wt[:, kc, :],
                                        start=(kc == 0), stop=False)
                                nc.tensor.matmul(
                                    acc[:], ones_bf[0:1, 0:ST],
                                    bias_slice(bias, cb, CB)[:],
                                    start=False, stop=True)
                                sq = nr.tile([ST, CB], F32, tag="sq")
                                nc.scalar.activation(
                                    sq[:], acc[:],
                                    mybir.ActivationFunctionType.Square,
                                    accum_out=var_p[:, st, cb:cb + 1])
                                nc.vector.tensor_copy(
                                    rows_all[:, st, cb * CB:(cb + 1) * CB], acc[:])
                        if mid is not None:
                            mid()
                        # pass B: rstd, normalize, gain, rope (in place)
                        for st in range(NST):
                            var = nr.tile([ST, 1], F32, tag="var")
                            nc.vector.reduce_sum(var[:], var_p[:, st, :],
                                                 axis=mybir.AxisListType.X)
                            sdt = nr.tile([ST, 1], F32, tag="sd")
                            nc.scalar.activation(
                                sdt[:], var[:], mybir.ActivationFunctionType.Sqrt,
                                bias=eps_c[0:ST, :], scale=rcpD_c[0:ST, :])
                            rstd = nr.tile([ST, 1], F32, tag="rstd")
                            nc.vector.reciprocal(rstd[:], sdt[:])
                            if kc_outer:
                                rows = nr.tile([ST, D], F32, tag=f"rows{st}",
                                               name=f"rows_{st}")
                            else:
                                rows = nr.tile([ST, D], F32, tag="rows")
                            nc.scalar.activation(
                                rows[:], rows_all[:, st, :],
                                mybir.ActivationFunctionType.Copy,
                                scale=rstd[:])
                            nc.vector.tensor_mul(rows[:], rows[:], g_b[0:ST, :])
                            # rope: out = rows*cosF + swap(rows)*sinF
                            swap = nr.tile([ST, D], F32, tag="swap")
                            r3 = rows[:].rearrange("p (h r two) -> p h r two",
                                                   two=2, r=ROT)
                            s3 = swap[:].rearrange("p (h r two) -> p h r two",
                                                   two=2, r=ROT)
                            nc.vector.tensor_copy(s3[:, :, :, 0], r3[:, :, :, 1])
                            nc.vector.tensor_copy(s3[:, :, :, 1], r3[:, :, :, 0])
                            cosb = (cs_sb[:, 0, st, :].unsqueeze(1)
                                    .broadcast_to((ST, H, HD)))
                            sinb = (cs_sb[:, 1, st, :].unsqueeze(1)
                                    .broadcast_to((ST, H, HD)))
                            rr = rows[:].rearrange("p (h f) -> p h f", h=H)
                            sr = swap[:].rearrange("p (h f) -> p h f", h=H)
                            nc.vector.tensor_mul(rr[:], rr[:], cosb)
                            nc.vector.tensor_mul(sr[:], sr[:], sinb)
                            nc.vector.tensor_add(rows[:], rows[:], swap[:])
                            if not kc_outer:
                                for kc in range(NKC):
                                    pt = ps_tp.tile([128, ST], F32, tag="tp")
                                    nc.tensor.transpose(
                                        pt[:], rows[:, kc * 128:(kc + 1) * 128],
                                        ident[0:ST, 0:ST])
                                    sink(kc, st, pt)
                            else:
                                rows_by_st.append(rows)
                        if kc_outer:
                            # head-major transpose order: head h complete
                            # after 4 transposes -> attention starts early
                            for kc in range(NKC):
                                for st in range(NST):
                                    pt = ps_tp.tile([128, ST], F32, tag="tp")
                                    nc.tensor.transpose(
                                        pt[:], rows_by_st[st][:, kc * 128:(kc + 1) * 128],
                                        ident[0:ST, 0:ST])
                                    sink(kc, st, pt)
                            rows_by_st.clear()

                rows_by_st = []

                qstage = {}

                def q_sink(kc, st, pt):
                    if st not in qstage:
                        qstage[st] = ktmp.tile([128, NKC, ST], BF16,
                                               tag="qtr", name=f"qstage{st}")
                    q_sb = qstage[st]
                    nc.vector.tensor_copy(q_sb[:, kc, :], pt[:])
                    if kc == NKC - 1:
                        nc.sync.dma_start(
                            out=q_bounce[:, st * ST:(st + 1) * ST]
                            .rearrange("(k p) s -> p k s", p=128),
                            in_=q_sb[:])
                        del qstage[st]

                ktmp_pool = tc.tile_pool(name="ktmp", bufs=3)
                ktmp = ktmp_pool.__enter__()

                kstage = {}

                def k_sink(kc, st, pt):
                    if st not in kstage:
                        kstage[st] = ktmp.tile([128, NKC, ST], BF16, tag="ktr", name=f"kstage{st}")
                    kt_sb = kstage[st]
                    nc.vector.tensor_copy(kt_sb[:, kc, :], pt[:])
                    if kc == NKC - 1:
                        nc.sync.dma_start(
                            out=kT_bounce[:, st * ST:(st + 1) * ST]
                            .rearrange("(k p) s -> p k s", p=128),
                            in_=kt_sb[:])
                        del kstage[st]

                def v_mid():
                    # v projection + its gathers run while k's pass B
                    # (vector-bound) executes
                    project(Wv, bv, v_out, CBV, NCBV)
                    for g in range(2):
                        nc.gpsimd.collective_compute(
                            "AllGather", mybir.AluOpType.bypass,
                            replica_groups=[list(range(NC))],
                            ins=[v_bounce[g].opt()], outs=[v_g[g][:].opt()])

                norm_rope(Wk, bk, gk_b, k_sink, mid=v_mid)
                for g in range(2):
                    nc.gpsimd.collective_compute(
                        "AllGather", mybir.AluOpType.bypass,
                        replica_groups=[list(range(NC))],
                        ins=[kT_bounce[g * DH2:(g + 1) * DH2, :].opt()],
                        outs=[kT_g[g][:].opt()])
                norm_rope(Wq, bq, gq_b, q_sink)
                ktmp_pool.__exit__(None, None, None)
                if DEBUG:
                    nc.sync.dma_start(
                        out=dbg_qT.rearrange("p h r -> (h p) r"), in_=q_bounce[:])
                    nc.sync.dma_start(out=dbg_gq[:], in_=gq_b[:])
                    nc.sync.dma_start(out=dbg_gk[:], in_=gk_b[:])
                    for g in range(2):
                        nc.sync.dma_start(out=dbg_kg[g][:], in_=kT_g[g][:])
                        nc.sync.dma_start(out=dbg_vg[g][:], in_=v_g[g][:])

            # ---- phase 4: attention ----
            with tc.tile_pool(name="late", bufs=1) as late:
              attnT = late.tile([128, H, ROWS], BF16)  # attention out^T
              with (
                tc.tile_pool(name="att", bufs=2) as att,
                tc.tile_pool(name="mk", bufs=1) as mk,
                tc.tile_pool(name="psO", bufs=1, space="PSUM") as psO,
                tc.tile_pool(name="ps_s", bufs=2, space="PSUM") as ps_s_pool,
            ):
                mask_sb = mk.tile([KT, max(N_MASKS, 1), 2 * CH], BF16)
                nc.sync.dma_start(out=mask_sb[:],
                                  in_=masks.rearrange("n p q -> p n q"))
                # mask index lookup: kt -> slot
                fused_slot = {kt: i for i, kt in enumerate(FUSED_MASK_KTS)}
                solo_slot = {kt: len(FUSED_MASK_KTS) + i
                             for i, kt in enumerate(SOLO_MASK_KTS)}

                # per-head loads and compute
                for h in range(H):
                    g, hh = h // HG, h % HG
                    kT_h = att.tile([128, NC, ROWS], BF16, tag="kTh")
                    nc.sync.dma_start(
                        out=kT_h[:],
                        in_=kT_g[g][:, hh * HD:(hh + 1) * HD, :]
                        .rearrange("s p c -> p s c"))
                    qT_h = att.tile([128, ROWS], BF16, tag="qTh2")
                    nc.sync.dma_start(out=qT_h[:],
                                      in_=q_bounce[h * HD:(h + 1) * HD, :])
                    v_h = att.tile([KT, NC * 4, HD], BF16, tag="vh")
                    for src in range(NC):
                        nc.sync.dma_start(
                            out=v_h[:, src * 4:(src + 1) * 4, :],
                            in_=v_g[g][src, :, hh * HD:(hh + 1) * HD]
                            .rearrange("(r p) d -> p r d", p=KT))

                    ps_o = psO.tile([128, 2, CH], F32, tag="o")
                    ps_den = psO.tile([1, 2, CH], F32, tag="den")
                    ps_o2 = psO.tile([128, CH], F32, tag="o2")
                    ps_den2 = psO.tile([1, CH], F32, tag="den2")
                    for kt in range(BND[1]):
                        src, coff = kt_src(kt)
                        vidx = src * 4 + (coff // KT)
                        if kt < BND[0]:
                            # fused: both query chunks, N = 504
                            ps_s = ps_s_pool.tile([KT, 2, CH], F32, tag="s")
                            nc.tensor.matmul(
                                ps_s[:], kT_h[:, src, coff:coff + KT],
                                qT_h[:], start=True, stop=True)
                            expT = att.tile([KT, 2, CH], BF16, tag="expT")
                            nc.scalar.activation(
                                expT[:], ps_s[:],
                                mybir.ActivationFunctionType.Exp,
                                scale=scl_c[0:KT, :])
                            slot = fused_slot.get(kt)
                            if slot is not None:
                                nc.vector.tensor_mul(
                                    expT[:], expT[:],
                                    mask_sb[:, slot, :].rearrange(
                                        "p (two q) -> p two q", two=2))
                            first, last = kt == 0, kt == BND[0] - 1
                            nc.tensor.matmul(
                                ps_den[:], ones_bf[0:KT, 0:1], expT[:],
                                start=first, stop=last)
                            nc.tensor.matmul(
                                ps_o[:], v_h[:, vidx, :], expT[:],
                                start=first, stop=last)
                        else:
                            # solo: only query chunk 1, N = 252
                            ps_s = ps_s_pool.tile([KT, 2, CH], F32, tag="s")
                            nc.tensor.matmul(
                                ps_s[:, 1, :], kT_h[:, src, coff:coff + KT],
                                qT_h[:, CH:2 * CH], start=True, stop=True)
                            expT = att.tile([KT, CH], BF16, tag="expT2")
                            nc.scalar.activation(
                                expT[:], ps_s[:, 1, :],
                                mybir.ActivationFunctionType.Exp,
                                scale=scl_c[0:KT, :])
                            slot = solo_slot.get(kt)
                            if slot is not None:
                                nc.vector.tensor_mul(
                                    expT[:], expT[:],
                                    mask_sb[:, slot, CH:2 * CH])
                            first, last = kt == BND[0], kt == BND[1] - 1
                            nc.tensor.matmul(
                                ps_den2[:], ones_bf[0:KT, 0:1], expT[:],
                                start=first, stop=last)
                            nc.tensor.matmul(
                                ps_o2[:], v_h[:, vidx, :], expT[:],
                                start=first, stop=last)

                    # finalize head: copy PSUM accumulators to SBUF fast
                    # (psO bufs=1 -- frees the banks for the next head),
                    # then den = fused + solo, recip, scale
                    den_sb = att.tile([1, 2, CH], F32, tag="densb")
                    nc.scalar.activation(den_sb[:], ps_den[:],
                                         mybir.ActivationFunctionType.Copy)
                    nc.vector.tensor_add(den_sb[:, 1, :], ps_den2[:],
                                         den_sb[:, 1, :])
                    o_sb = att.tile([128, 2, CH], F32, tag="osb")
                    nc.scalar.activation(o_sb[:], ps_o[:],
                                         mybir.ActivationFunctionType.Copy)
                    nc.vector.tensor_add(o_sb[:, 1, :], ps_o2[:],
                                         o_sb[:, 1, :])
                    rd = att.tile([1, 2, CH], F32, tag="rd")
                    nc.vector.reciprocal(rd[:], den_sb[:])
                    nc.sync.dma_start(out=recip_scratch[h, None, :, :],
                                      in_=rd[:])
                    rb = att.tile([128, 2, CH], F32, tag="rb")
                    nc.sync.dma_start(
                        out=rb[:],
                        in_=recip_scratch[h, None, :, :].partition_broadcast(128))
                    nc.vector.tensor_mul(o_sb[:], o_sb[:], rb[:])
                    nc.vector.tensor_copy(
                        attnT[:, h, :].rearrange("p (two q) -> p two q", two=2),
                        o_sb[:])
                if DEBUG:
                    nc.sync.dma_start(out=dbg_mask[:], in_=mask_sb[:])

              if DEBUG:
                  nc.sync.dma_start(out=dbg_attnT[:], in_=attnT[:])
                  nc.sync.dma_start(out=dbg_recip[:], in_=recip_scratch[:])


              # ---- phase 5: output projection ----
              with (
                  tc.tile_pool(name="op", bufs=3) as op,
                  tc.tile_pool(name="ps_op", bufs=2, space="PSUM") as ps_op,
              ):
                  for eb in range(NCB):
                      wt = op.tile([128, NKC, CB], BF16, tag="wo")
                      nc.sync.dma_start(
                          out=wt[:],
                          in_=Wo[:, eb * CB:(eb + 1) * CB]
                          .rearrange("(k p) c -> p k c", p=128))
                      for st in range(NST):
                          acc = ps_op.tile([ST, CB], F32, tag=f"acc{st % 2}")
                          for ct in range(NKC):
                              nc.tensor.matmul(
                                  acc[:],
                                  attnT[:, ct, st * ST:(st + 1) * ST],
                                  wt[:, ct, :],
                                  start=(ct == 0), stop=False)
                          bo_t = op.tile([1, CB], F32, tag="bo")
                          nc.sync.dma_start(
                              out=bo_t[:].bitcast(F32R),
                              in_=bo[None, eb * CB:(eb + 1) * CB].bitcast(F32R))
                          bo16 = op.tile([1, CB], BF16, tag="bo16")
                          nc.scalar.activation(bo16[:], bo_t[:],
                                               mybir.ActivationFunctionType.Copy)
                          nc.tensor.matmul(
                              acc[:], ones_bf[0:1, 0:ST],
                              bo16[:],
                              start=False, stop=True)
                          ot = op.tile([ST, CB], F32, tag="orow")
                          nc.scalar.activation(ot[:], acc[:],
                                               mybir.ActivationFunctionType.Copy)
                          nc.sync.dma_start(
                              out=out_own[st * ST:(st + 1) * ST,
                                          eb * CB:(eb + 1) * CB],
                              in_=ot[:])

    fix_waits(nc)
    return nc


# ---------------- host side ----------------

def _rope_cos_sin():
    """Static index maps for rope angle rows; returns function of freqs."""
    t_dim = ROT - 2 * (ROT // 3)   # 22
    s1 = ROT // 3                  # 21
    idx = np.arange(S)
    f_idx = idx // FRAME
    h_idx = (idx // W_GRID) % H_GRID
    w_idx = idx % W_GRID
    def build(freqs):
        ang = np.empty((S, ROT), np.float32)
        ang[:, :t_dim] = freqs[f_idx, :t_dim]
        ang[:, t_dim:t_dim + s1] = freqs[h_idx, t_dim:t_dim + s1]
        ang[:, t_dim + s1:] = freqs[w_idx, t_dim + s1:]
        cos = np.cos(ang).astype(np.float32)
        sin = np.sin(ang).astype(np.float32)
        cosf = np.repeat(cos, 2, axis=1)                     # [S, 128]
        sinf = np.empty((S, HD), np.float32)
        sinf[:, 0::2] = -sin
        sinf[:, 1::2] = sin
        return cosf, sinf
    return build

_build_cos_sin = _rope_cos_sin()


def _host_masks(c):
    """Mask tiles [N_MASKS, KT, 2*CH] for core c (1.0 allowed).
    Fused slots cover kt in FUSED_MASK_KTS with both query chunks
    (half 0 = chunk ja, half 1 = chunk jb); solo slots cover kt in
    SOLO_MASK_KTS with only chunk jb (half 1)."""
    frame_of = np.arange(S) // FRAME
    ja, jb = core_chunks(c)
    qf = [frame_of[chunk_rows(ja)[0]:chunk_rows(ja)[1]],
          frame_of[chunk_rows(jb)[0]:chunk_rows(jb)[1]]]
    out = np.ones((max(N_MASKS, 1), KT, 2, CH), np.float32)
    slot = 0
    for kt in FUSED_MASK_KTS:
        kf = frame_of[KT * kt:KT * (kt + 1)]
        out[slot, :, 0, :] = (kf[:, None] <= qf[0][None, :])
        out[slot, :, 1, :] = (kf[:, None] <= qf[1][None, :])
        slot += 1
    for kt in SOLO_MASK_KTS:
        kf = frame_of[KT * kt:KT * (kt + 1)]
        out[slot, :, 1, :] = (kf[:, None] <= qf[1][None, :])
        slot += 1
    import ml_dtypes
    return out.reshape(max(N_MASKS, 1), KT, 2 * CH).astype(ml_dtypes.bfloat16)


_CACHE = {}


def _get_program():
    if "nc" not in _CACHE:
        _CACHE["nc"] = build_program()
    return _CACHE["nc"]


def kernel(**inputs):
    from concourse.bass_utils import run_bass_kernel_spmd

    x = np.asarray(inputs["x"], np.float32)       # [1, S, D]
    freqs = np.asarray(inputs["freqs"], np.float32)
    cosf, sinf = _build_cos_sin(freqs)

    import ml_dtypes
    common = {k: np.ascontiguousarray(np.asarray(inputs[k], np.float32))
              for k in ("bq", "bk", "bv", "bo", "gq", "gk")}
    for k in ("Wq", "Wk", "Wv", "Wo"):
        common[k] = np.ascontiguousarray(
            np.asarray(inputs[k], np.float32).astype(ml_dtypes.bfloat16))

    in_maps = []
    for c in range(NC):
        ja, jb = core_chunks(c)
        rows = np.concatenate([
            x[0, CH * ja:CH * (ja + 1)], x[0, CH * jb:CH * (jb + 1)]])
        cs = np.concatenate([
            cosf[CH * ja:CH * (ja + 1)], cosf[CH * jb:CH * (jb + 1)]])
        sn = np.concatenate([
            sinf[CH * ja:CH * (ja + 1)], sinf[CH * jb:CH * (jb + 1)]])
        m = {"x_own": np.ascontiguousarray(rows),
             "cosf": np.ascontiguousarray(cs),
             "sinf": np.ascontiguousarray(sn),
             "masks": _host_masks(c)}
        m.update(common)
        in_maps.append(m)

    nc = _get_program()
    res = run_bass_kernel_spmd(nc, in_maps, list(range(NC)))

    out = np.empty((1, S, D), np.float32)
    for c in range(NC):
        ja, jb = core_chunks(c)
        r = res.results[c]["out_own"]
        out[0, CH * ja:CH * (ja + 1)] = r[:CH]
        out[0, CH * jb:CH * (jb + 1)] = r[CH:]
    return out
